# revision 1
# baseline (speedup 1.0000x reference)
"""Bass/Trainium2 kernel for nn_AvgPoolBackbone (segment_reduce).

Computes, for each batch row b of x [B, S, D]:
    eff = S if idx[b] == -1 else idx[b]
    out[b] = mean(x[b, :eff], axis=0)   (zeros when eff <= 0)

Strategy
--------
Pure data parallel over 8 NeuronCores (16 batches each).  On the host we
fold the prefix mask AND the 1/eff_len scaling into a single f32 matrix
`maskt` (maskt[p, b*16+k] = (p*16+k < eff[b]) / max(eff[b], 1)) so the
device does no division and no control flow; the masked mean is just a
weighted reduction over the sequence axis.

Per batch, x[b] ([2048, 256] f32, 2 MiB) is DMA'd as [128, 16*256]:
partition p holds the 16 consecutive sequence rows p*16..p*16+15 — one
contiguous 16 KiB DRAM run per partition, which keeps the 16 SDMA
engines at line rate (~435 GB/s aggregate; the kernel is HBM/fabric
bound at ~80 us per core).  One 2 MiB DMA per batch on the sync HWDGE
ring, in consumption order, double-buffered 6 deep.

fp32 TensorE matmuls pay a 2-pass penalty (4 cycles/output element), so
a single engine cannot keep up with the DMA stream in exact fp32.  Each
batch is therefore split across two engines working in parallel:

 - VectorE: 6 of the 16 d-row-slices via a fused multiply-accumulate
   chain, acc_sb[128, d] (+)= x_slice * mask_col
   (scalar_tensor_tensor, per-partition scalar = scaled mask column)
 - TensorE: the other 10 slices as PSUM-accumulated matmuls
   psum[1, d] += mask_col.T @ x_slice, plus one "ones" matmul that
   folds acc_sb across partitions into the same PSUM group.  The fold
   is deferred until the NEXT batch's matmuls are emitted so TensorE
   never stalls at the head of a fresh DVE chain.
 - ScalarE: PSUM -> SBUF result copies (and the small mask-matrix DMA,
   on its own HWDGE ring so the x stream starts immediately).

All arithmetic is exact fp32 (measured rel err vs the f32 reference
~4e-7).  Measured ~101 us per core on TRN2 against a ~80 us DMA floor.
"""

import numpy as np

import concourse.bass as bass
import concourse.tile as tile
from concourse import bacc, mybir
from concourse import bass_utils

F32 = mybir.dt.float32
F32R = mybir.dt.float32r

# Problem config (hardcoded per the harness contract).
B, S, D = 128, 2048, 256
N_CORES = 8
BL = B // N_CORES  # batches per core
P = 128            # SBUF partitions


def build_kernel(bl=BL, s=S, d=D, f32r=False, split=True, bufs=6, q16=6, g=0, pe_first=False):
    """Build + compile the single-core Bass module (same NEFF on all cores).

    split=True: every batch is split DVE/PE as described in the module
    docstring (exact fp32).  split=False with f32r=True instead runs
    everything on PE in reduced-precision float32r (single-pass matmuls;
    ~5 us faster but ~1.5e-4 rel err).  q16: sixteenths of each batch
    handled by the DVE chain.
    """
    j = s // P  # seq rows per partition (16 at full size)
    mmdt = F32R if f32r else F32
    if f32r:
        split = False
    q = q16 * j // 16  # j-slices per batch on DVE in split mode
    nc = bacc.Bacc("TRN2", target_bir_lowering=False, debug=False)
    x = nc.dram_tensor("x", (bl, s, d), mmdt, kind="ExternalInput")
    maskt = nc.dram_tensor("maskt", (P, bl * j), mmdt, kind="ExternalInput")
    out = nc.dram_tensor("out", (1, bl * d), F32, kind="ExternalOutput")

    with tile.TileContext(nc) as tc:
        with (
            tc.tile_pool(name="xp", bufs=bufs) as xp,
            tc.tile_pool(name="xtp", bufs=1) as xtp,
            tc.tile_pool(name="mp", bufs=1) as mp,
            tc.tile_pool(name="op", bufs=1) as op,
            tc.tile_pool(name="ap", bufs=6) as apool,
            tc.tile_pool(name="ps", bufs=8, space=bass.MemorySpace.PSUM) as ps,
        ):
            m_t = mp.tile([P, bl * j], mmdt)
            # mask load on the scalar HWDGE ring so the sync ring's x
            # stream starts immediately; lands well before first use
            nc.scalar.dma_start(m_t[:], maskt.ap())
            ones_t = None
            if split:
                ones_t = mp.tile([P, 1], F32)
                nc.vector.memset(ones_t[:], 1.0)
            o_t = op.tile([1, bl * d], F32)
            xv = x.ap().rearrange("b (p k) d -> p b (k d)", p=P)

            def dve_chain(b, acc_sb, jis, eng=None):
                eng = eng or nc.vector
                for n, ji in enumerate(jis):
                    xs = x_tiles[b][:, ji * d : (ji + 1) * d]
                    mcol = m_t[:, b * j + ji : b * j + ji + 1]
                    if n == 0:
                        eng.tensor_scalar_mul(acc_sb[:], xs, mcol)
                    else:
                        eng.scalar_tensor_tensor(
                            acc_sb[:],
                            xs,
                            mcol,
                            acc_sb[:],
                            mybir.AluOpType.mult,
                            mybir.AluOpType.add,
                        )

            def pe_mms(b, acc, jis, start, stop):
                for n, ji in enumerate(jis):
                    nc.tensor.matmul(
                        acc[:],
                        m_t[:, b * j + ji : b * j + ji + 1],
                        x_tiles[b][:, ji * d : (ji + 1) * d],
                        start=(start and n == 0),
                        stop=(stop and n == len(jis) - 1),
                    )

            def emit_fold(pb, paccs, pacc):
                for n, a in enumerate(paccs):
                    nc.tensor.matmul(
                        pacc[:], ones_t[:], a[:],
                        start=False, stop=(n == len(paccs) - 1),
                    )
                nc.scalar.copy(o_t[:, pb * d : (pb + 1) * d], pacc[:])

            x_tiles = {}
            pending = None  # (batch, acc_sb, acc) awaiting its fold matmul
            for b in range(bl):
                # one 2 MiB DMA per batch on the sync HWDGE ring, in
                # consumption order; lands as [P, j*d] with one contiguous
                # 16 KiB DRAM run per partition.  The two tail batches get
                # dedicated SBUF slots so their DMAs never wait on a slot
                # release gated by late compute.
                if b >= bl - 2:
                    x_t = xtp.tile([P, j * d], mmdt, tag=f"xtail{b}")
                else:
                    x_t = xp.tile([P, j * d], mmdt)
                nc.sync.dma_start(x_t[:], xv[:, b])
                x_tiles[b] = x_t
                if b == bl - 1:
                    # first half of the output ships while the tail computes
                    nc.sync.dma_start(
                        out.ap()[:, : bl * d // 2], o_t[:, : bl * d // 2]
                    )
                if split:
                    acc_sb = apool.tile([P, d], F32)
                    acc = ps.tile([1, d], F32)
                    if pe_first:
                        pe_mms(b, acc, range(q, j), start=True, stop=False)
                        dve_chain(b, acc_sb, range(q))
                    else:
                        dve_chain(b, acc_sb, range(q))
                        pe_mms(b, acc, range(q, j), start=True, stop=False)
                    if pending is not None:
                        emit_fold(*pending)
                    pending = (b, [acc_sb], acc)
                else:
                    acc = ps.tile([1, d], F32)
                    pe_mms(b, acc, range(j), start=True, stop=True)
                    nc.scalar.copy(o_t[:, b * d : (b + 1) * d], acc[:])
            if pending is not None:
                emit_fold(*pending)
            nc.sync.dma_start(
                out.ap()[:, bl * d // 2 :], o_t[:, bl * d // 2 :]
            )

    nc.compile()
    return nc


def make_host_inputs(x, start_padding_indices, n_cores=N_CORES, bl=BL, s=S, d=D):
    """Shard x and build the per-core scaled mask matrices.

    maskt[p, b*j + ji] = (p*j + ji < eff[b]) / max(eff[b], 1)
    """
    x = np.ascontiguousarray(np.asarray(x, dtype=np.float32))
    idx = np.asarray(start_padding_indices).astype(np.int64)
    j = s // P
    eff = np.where(idx == -1, s, idx).astype(np.int64)  # [B]
    scale = 1.0 / np.maximum(eff, 1).astype(np.float64)
    mask = (np.arange(s)[None, :] < eff[:, None]) * scale[:, None]  # [B, S] f64
    mask = mask.astype(np.float32)
    # [B, S] -> [B, P, j] (s-major within partition) -> cores pack [P, bl*j]
    mask_pj = mask.reshape(-1, P, j)  # [B, P, j]
    in_maps = []
    for c in range(n_cores):
        mb = mask_pj[c * bl : (c + 1) * bl]  # [bl, P, j]
        maskt = np.ascontiguousarray(mb.transpose(1, 0, 2).reshape(P, bl * j))
        in_maps.append(
            {
                "x": np.ascontiguousarray(x[c * bl : (c + 1) * bl]),
                "maskt": maskt,
            }
        )
    return in_maps


_CACHED_NC = None


def _get_nc():
    global _CACHED_NC
    if _CACHED_NC is None:
        _CACHED_NC = build_kernel()
    return _CACHED_NC


def run(x, start_padding_indices, trace=False):
    """Run on all 8 cores; returns (out [B, D] f32, BassKernelResults)."""
    nc = _get_nc()
    in_maps = make_host_inputs(x, start_padding_indices)
    res = bass_utils.run_bass_kernel_spmd(
        nc, in_maps, core_ids=list(range(N_CORES)), trace=trace
    )
    outs = [r["out"].reshape(BL, D) for r in res.results]
    return np.concatenate(outs, axis=0), res


def kernel(x, start_padding_indices):
    out, _ = run(x, start_padding_indices, trace=False)
    return out



# revision 2
# speedup vs baseline: 1.1178x; 1.1178x over previous
"""Bass/Trainium2 kernel for nn_AvgPoolBackbone (segment_reduce), packed.

Computes, for each batch row b of x [B, S, D]:
    eff = S if idx[b] == -1 else idx[b]
    out[b] = mean(x[b, :eff], axis=0)   (zeros when eff <= 0)

Strategy
--------
Rows past eff[b] contribute nothing, so the kernel never reads them.
The host packs each batch's valid prefix (rounded up to 16-row groups;
the tail rows of the last, partial group are left zero) into one dense
stream and splits it evenly across the 8 cores at group granularity
(batches may straddle a core boundary; their two partial sums are added
on the host).  Per core that is ~18 MB of HBM traffic instead of
33.5 MB — the kernel is purely DMA-bound, so this is the big lever.

Device layout: the per-core stream is a sequence of 2 MiB chunks
[128 partitions x 16 rows x 256], partition p holding 16 consecutive
stream rows (one 16 KiB contiguous DRAM run per partition).  Because
the host zeroes invalid rows, every partition's 16 rows share one
weight (1/eff of its batch, or 0 for padding), so the whole reduction
collapses onto TensorE: per chunk, 8 float32r matmuls

    psum[NB, 512] += selw_t.T @ x_t[:, j*512:(j+1)*512]

with selw[p, slot] = 1/eff (the weighted partition->batch-slot selector
built on the host) as the shared stationary matrix.  Each matmul folds
two row-slices at once (moving free dim 512 = 2 x D); all matmuls of
all chunks accumulate into a single [NB, 512] PSUM group.  At the end
one DVE add folds the two 256-halves and the [NB, 256] result ships
out.  DVE/ACT/GpSimd are otherwise idle; the DMA x-stream never waits
on compute (every chunk has its own SBUF slot).

float32r matmuls are single-pass (reduced-precision fp32, rel err
~1e-5 here vs the fp32 reference, tolerance is 2e-2).
"""

import numpy as np

import concourse.bass as bass
import concourse.tile as tile
from concourse import bacc, mybir
from concourse import bass_utils

F32 = mybir.dt.float32
F32R = mybir.dt.float32r

# Problem config (hardcoded per the harness contract).
B, S, D = 128, 2048, 256
N_CORES = 8
P = 128            # SBUF partitions
GRP = 16           # stream rows per group (one partition's rows per chunk)
CHUNK_G = 128      # groups per chunk (= 2048 rows = 2 MiB)
MMF = 512          # moving free dim per matmul (2 slices of D)


def plan_shards(start_padding_indices):
    idx = np.asarray(start_padding_indices).astype(np.int64).reshape(-1)
    eff = np.where(idx == -1, S, np.clip(idx, 0, S)).astype(np.int64)
    gb = (eff + GRP - 1) // GRP          # 16-row groups per batch
    g_total = int(gb.sum())
    gt = max(-(-g_total // N_CORES), 1)  # groups per core
    cum = np.concatenate([[0], np.cumsum(gb)])
    cores = []
    for c in range(N_CORES):
        lo, hi = c * gt, (c + 1) * gt
        segs = []   # (batch, first group within batch, n groups, dst group)
        for b in range(B):
            s0, s1 = int(cum[b]), int(cum[b + 1])
            o0, o1 = max(s0, lo), min(s1, hi)
            if o0 < o1:
                segs.append((b, o0 - s0, o1 - o0, o0 - lo))
        cores.append(segs)
    nb = max(max((len(s) for s in cores), default=1), 1)
    return eff, gt, nb, cores


def make_host_inputs(x, eff, gt, nb, cores):
    x = np.asarray(x, dtype=np.float32)
    t_chunks = -(-gt // CHUNK_G)
    gt_pad = t_chunks * CHUNK_G
    in_maps, slot_maps = [], []
    for segs in cores:
        xc = np.zeros((gt * GRP, D), dtype=np.float32)
        sv = np.zeros((gt_pad, nb), dtype=np.float32)
        slots = []
        for i, (b, g0, gc, dst) in enumerate(segs):
            slots.append(b)
            r0, r1 = g0 * GRP, (g0 + gc) * GRP
            r1v = min(r1, int(eff[b]))   # only valid rows; group tail stays 0
            if r1v > r0:
                xc[dst * GRP : dst * GRP + (r1v - r0)] = x[b, r0:r1v]
            sv[dst : dst + gc, i] = 1.0 / max(float(eff[b]), 1.0)
        # [gt_pad, nb] -> [T, 128, nb] -> [128, T*nb]  (partition-major)
        st = sv.reshape(t_chunks, CHUNK_G, nb).transpose(1, 0, 2)
        in_maps.append(
            {
                "x": np.ascontiguousarray(xc.reshape(gt, GRP * D)),
                "selw": np.ascontiguousarray(st.reshape(CHUNK_G, t_chunks * nb)),
            }
        )
        slot_maps.append(slots)
    return in_maps, slot_maps


def build_kernel(gt, nb):
    """Single-core Bass module (same NEFF on all cores)."""
    t_chunks = -(-gt // CHUNK_G)
    n_full = gt // CHUNK_G
    g_r = gt % CHUNK_G
    n_mmpc = (GRP * D) // MMF            # matmuls per chunk (8)
    total_mm = t_chunks * n_mmpc

    nc = bacc.Bacc("TRN2", target_bir_lowering=False, debug=False)
    x = nc.dram_tensor("x", (gt, GRP * D), F32R, kind="ExternalInput")
    selw = nc.dram_tensor("selw", (P, t_chunks * nb), F32R, kind="ExternalInput")
    out = nc.dram_tensor("out", (nb, D), F32, kind="ExternalOutput")

    # Process the (short) remainder chunk first so the stream ends on a
    # full, engine-balanced chunk; split every chunk's DMA into pieces
    # (halves; quarters for the final chunk) with one SBUF tile each so
    # the matmuls chase the stream at sub-chunk granularity and the
    # post-stream tail is just a quarter-chunk of matmuls.
    # remainder chunk second: not last (it would leave the stream tail
    # partition-imbalanced) and not first (its partial partition set
    # leaves some DMA engines idle at stream start)
    order = list(range(n_full))
    if g_r:
        order.insert(min(1, len(order)), t_chunks - 1)
    pieces = []
    for oi, t in enumerate(order):
        pc = g_r if (g_r and t == t_chunks - 1) else CHUNK_G
        ksp = (0, 4, 8, 12, GRP) if oi == len(order) - 1 else (0, 8, GRP)
        for k0, k1 in zip(ksp[:-1], ksp[1:]):
            pieces.append((t, pc, k0, k1))

    with tile.TileContext(nc) as tc:
        with (
            tc.tile_pool(name="xp", bufs=1) as xp,
            tc.tile_pool(name="mp", bufs=1) as mp,
            tc.tile_pool(name="op", bufs=1) as op,
            tc.tile_pool(name="ps", bufs=1, space=bass.MemorySpace.PSUM) as ps,
        ):
            s_t = mp.tile([P, t_chunks * nb], F32R, tag="selw")
            # selector load on the scalar HWDGE ring so the sync ring's
            # x stream starts immediately
            nc.scalar.dma_start(s_t[:], selw.ap())
            o_t = op.tile([nb, D], F32)
            acc_ps = ps.tile([nb, MMF], F32)
            xv = x.ap()
            mm = 0
            for idx, (t, pc, k0, k1) in enumerate(pieces):
                x_t = xp.tile(
                    [pc, (k1 - k0) * D], F32R, tag=f"x{idx}", name=f"x{idx}"
                )
                nc.sync.dma_start(
                    x_t[:],
                    xv[t * CHUNK_G : t * CHUNK_G + pc, k0 * D : k1 * D],
                )
                for jl in range((k1 - k0) * D // MMF):
                    nc.tensor.matmul(
                        acc_ps[:],
                        s_t[:pc, t * nb : (t + 1) * nb],
                        x_t[:, jl * MMF : (jl + 1) * MMF],
                        start=(mm == 0),
                        stop=(mm == total_mm - 1),
                    )
                    mm += 1
            # fold the two 256-halves and ship out on the idle scalar ring
            nc.vector.tensor_scalar_mul(o_t[:], acc_ps[:, :D], 1.0)
            nc.vector.scalar_tensor_tensor(
                o_t[:],
                acc_ps[:, D:],
                1.0,
                o_t[:],
                mybir.AluOpType.mult,
                mybir.AluOpType.add,
            )
            nc.sync.dma_start(out.ap(), o_t[:])

    nc.compile()
    return nc


_CACHED = {}


def _get_nc(gt, nb):
    key = (gt, nb)
    if key not in _CACHED:
        _CACHED[key] = build_kernel(gt, nb)
    return _CACHED[key]


def run(x, start_padding_indices, trace=False):
    eff, gt, nb, cores = plan_shards(start_padding_indices)
    in_maps, slot_maps = make_host_inputs(x, eff, gt, nb, cores)
    nc = _get_nc(gt, nb)
    res = bass_utils.run_bass_kernel_spmd(
        nc, in_maps, core_ids=list(range(N_CORES)), trace=trace
    )
    out_full = np.zeros((B, D), dtype=np.float32)
    for c in range(N_CORES):
        oc = res.results[c]["out"].reshape(nb, D)
        for i, b in enumerate(slot_maps[c]):
            out_full[b] += oc[i]
    return out_full, res


def kernel(x, start_padding_indices):
    out, _ = run(x, start_padding_indices, trace=False)
    return out
